# revision 1
# baseline (speedup 1.0000x reference)
"""Trainium2 Bass kernel for nn_MAB_72911364817388 (dense transformer block).

Reference computation (per batch element b):
    q = Q @ Wq + bq ; k = K @ Wk + bk ; v = K @ Wv + bv        (1024x512 @ 512x512)
    scores = einsum("qhd,khd->hqk", qh, kh) / sqrt(512)
    scores = where(mask==0, -1e4, scores); attn = softmax(scores, axis=k)
    oh = qh + attn @ vh ; O = LN0(oh) ; O = O + relu(O @ Wo + bo) ; O = LN1(O)

Strategy: pure data-parallel over batch B=8 -> one batch element per core.
All on-chip activations are kept "d-major" (feature dim on partitions),
which makes attention, the FC layer and per-feature bias/scale natural.
LayerNorm reductions over d (the partition axis) are done with ones-vector
matmuls on the TensorEngine, which also replicates the stats across all
128 partitions for free.

Softmax is computed unnormalized: p = exp(s/sqrt(512) + maskbias), where
maskbias is -100 for masked keys (exp underflows to ~3e-44, matching the
reference's -1e4 masking after normalization to < 1e-40 relative).  The
denominator comes from an extra ones-column matmul and is divided out
after attn @ v.  Scores never exceed ~±6 so no max-subtraction is needed.

Matmuls run as float32r (fp32 storage, reduced-precision PE mode, 4x
faster than plain fp32; ~1.6e-4 rel err per matmul measured on HW).  The
attention core (scores and attn@v) uses bf16 operands, which stream at
the same rate but with much cheaper self-loading weights (~237ns vs
~329ns per 512-wide matmul measured).  Keys are host-compacted: unmasked
keys are moved to the front (softmax is permutation-invariant; fully
masked keys contribute exactly zero), so the attention core processes
~640 of 1024 keys for the usual p=0.5 masks.  Measured end-to-end
relative error vs the jax reference: ~6e-4 (scale-relative max).
"""

import numpy as np

import concourse.bass as bass
import concourse.mybir as mybir
import concourse.tile as tile
from concourse import bacc, bass_utils
from concourse.masks import make_identity

# Problem shapes (hardcoded per contract).
B = 8
NQ = NK = 1024
D = 512  # DQ = DK = DV
H = 8
HD = 64
P = 128
EPS = 1e-5
N_CORES = 8

DO = D // P  # 4   d-major partition groups
NO = NQ // P  # 8  q/k-major partition groups
QC = NQ // 512  # 2 free-dim chunks of 512

F32 = mybir.dt.float32
BF16 = mybir.dt.bfloat16
MM_DT = mybir.dt.float32r

AF = mybir.ActivationFunctionType
OP = mybir.AluOpType


def _mm(a):
    """Bitcast an fp32 AP to the matmul dtype (consumer side)."""
    return a.bitcast(MM_DT) if MM_DT != F32 else a


def _mo(a):
    """Bitcast a producer's out AP to the matmul dtype, so the BIR verifier
    sees matmul inputs as produced-rounded fp32r."""
    return a.bitcast(MM_DT) if MM_DT != F32 else a


def build_program(repeat: int = 1, apply_g0b0: bool = True,
                  apply_g1b1: bool = True, nkb: int = NO):
    """Build the per-core Bass program for nkb 128-row key blocks.

    The host compacts unmasked keys to the front (softmax is permutation-
    invariant over keys, and fully-masked keys contribute exactly 0), so
    nkb is usually ceil(max_unmasked/128) ~ 5 instead of 8."""
    nc = bacc.Bacc("TRN2", target_bir_lowering=False, debug=False,
                   num_devices=N_CORES)

    NKC = nkb * P
    Qd = nc.dram_tensor("Q", [NQ, D], F32, kind="ExternalInput").ap()
    Kd = nc.dram_tensor("K", [NKC, D], F32, kind="ExternalInput").ap()
    Md = nc.dram_tensor("attention_mask", [NKC], mybir.dt.int32,
                        kind="ExternalInput").ap()
    Wqd = nc.dram_tensor("Wq", [D, D], F32, kind="ExternalInput").ap()
    Wkd = nc.dram_tensor("Wk", [D, D], F32, kind="ExternalInput").ap()
    Wvd = nc.dram_tensor("Wv", [D, D], F32, kind="ExternalInput").ap()
    Wod = nc.dram_tensor("Wo", [D, D], F32, kind="ExternalInput").ap()
    bqd = nc.dram_tensor("bq", [D], F32, kind="ExternalInput").ap()
    bkd = nc.dram_tensor("bk", [D], F32, kind="ExternalInput").ap()
    bvd = nc.dram_tensor("bv", [D], F32, kind="ExternalInput").ap()
    bod = nc.dram_tensor("bo", [D], F32, kind="ExternalInput").ap()
    g0d = nc.dram_tensor("g0", [D], F32, kind="ExternalInput").ap()
    b0d = nc.dram_tensor("b0", [D], F32, kind="ExternalInput").ap()
    g1d = nc.dram_tensor("g1", [D], F32, kind="ExternalInput").ap()
    b1d = nc.dram_tensor("b1", [D], F32, kind="ExternalInput").ap()
    # Output is O^T (d-major); the host transposes back.
    OTd = nc.dram_tensor("OT", [D, NQ], F32, kind="ExternalOutput").ap()

    with tile.TileContext(nc) as tc:
        def body():
            _build_body(nc, tc,
                        Qd, Kd, Md, Wqd, Wkd, Wvd, Wod,
                        bqd, bkd, bvd, bod, g0d, b0d, g1d, b1d, OTd,
                        apply_g0b0, apply_g1b1, nkb)

        if repeat == 1:
            body()
        else:
            with tc.For_i(0, repeat, 1,
                          hint_engines=(mybir.EngineType.PE,
                                        mybir.EngineType.Activation,
                                        mybir.EngineType.DVE,
                                        mybir.EngineType.SP,
                                        mybir.EngineType.Pool)):
                body()

    nc.compile()
    return nc


def _build_body(nc, tc, Qd, Kd, Md, Wqd, Wkd, Wvd, Wod,
                bqd, bkd, bvd, bod, g0d, b0d, g1d, b1d, OTd,
                apply_g0b0, apply_g1b1, nkb):
    f32 = F32
    NKC = nkb * P
    kchunks = []
    off = 0
    while off < NKC:
        w = min(512, NKC - off)
        kchunks.append((off, w))
        off += w
    import contextlib
    ctx = contextlib.ExitStack()
    with ctx:
        consts = ctx.enter_context(tc.tile_pool(name="consts", bufs=1))
        bigs = ctx.enter_context(tc.tile_pool(name="bigs", bufs=1))
        small = ctx.enter_context(tc.tile_pool(name="small", bufs=3))

        # ---------- constants ----------
        ident = consts.tile([P, P], f32)
        make_identity(nc, ident)
        ones_stage = consts.tile([P, 512], f32)
        nc.vector.memset(ones_stage, 1.0)
        ones_pp = consts.tile([P, P], f32)   # all-ones for LN stat matmuls
        nc.vector.tensor_copy(out=_mo(ones_pp), in_=ones_stage[:, :P])
        ones_row = consts.tile([1, 512], f32)  # ones moving-vector for bias rank-1
        nc.vector.tensor_copy(out=_mo(ones_row), in_=ones_stage[:1, :])
        epsT = consts.tile([P, 1], f32)
        nc.vector.memset(epsT, EPS)

        # ---------- phase A: load + transpose Q, K; weight DMAs interleave
        Wq = consts.tile([P, DO, D], f32)
        Wk = consts.tile([P, DO, D], f32)
        Wv = consts.tile([P, DO, D], f32)
        Wo = consts.tile([P, DO, D], f32)
        QT = bigs.tile([P, DO, NQ], f32, tag="buf_qt")
        KT = bigs.tile([P, DO, NKC], f32, tag="buf_kt")
        with tc.tile_pool(name="ps_tr", bufs=4, space="PSUM") as ps_tr, \
             tc.tile_pool(name="rawp", bufs=3) as rawp:
            # batched raw loads: fewer, bigger DMAs (latency amortized)
            q0 = rawp.tile([P, 4, D], f32, tag="qraw")
            nc.sync.dma_start(out=q0,
                              in_=Qd[:512, :].rearrange("(j p) d -> p j d", p=P))
            nc.sync.dma_start(out=_mo(Wq),
                              in_=_mo(Wqd.rearrange("(o p) n -> p o n", p=P)))
            kr = rawp.tile([P, nkb, D], f32, tag="kraw")
            nc.sync.dma_start(out=kr,
                              in_=Kd.rearrange("(j p) d -> p j d", p=P))
            q1 = rawp.tile([P, 4, D], f32, tag="qraw")
            nc.sync.dma_start(out=q1,
                              in_=Qd[512:, :].rearrange("(j p) d -> p j d", p=P))
            nc.sync.dma_start(out=_mo(Wk),
                              in_=_mo(Wkd.rearrange("(o p) n -> p o n", p=P)))
            nc.sync.dma_start(out=_mo(Wv),
                              in_=_mo(Wvd.rearrange("(o p) n -> p o n", p=P)))
            nc.sync.dma_start(out=_mo(Wo),
                              in_=_mo(Wod.rearrange("(o p) n -> p o n", p=P)))

            worklist = [(j, 0, q0) for j in range(4)]
            worklist += [(j, 1, kr) for j in range(nkb)]
            worklist += [(j + 4, 0, q1) for j in range(4)]
            for no, sd, rawt in worklist:
                dstT, ceng = ((QT, 0), (KT, 1))[sd]
                raw = rawt[:, no % 4 if sd == 0 else no, :]
                ps = ps_tr.tile([P, 512], f32, tag="trps")
                for do in range(DO):
                    nc.tensor.transpose(ps[:, do * P:(do + 1) * P],
                                        raw[:, do * P:(do + 1) * P], ident)
                dst = dstT[:, :, no * P:(no + 1) * P]
                psv = ps.rearrange("p (a b) -> p a b", b=P)
                if (no + ceng) % 2 == 0:
                    nc.vector.tensor_copy(out=_mo(dst), in_=psv)
                else:
                    nc.scalar.copy(out=_mo(dst), in_=psv)

        # bias rows [1, 512]
        def load_row(ap):
            t = consts.tile([1, 512], f32)
            nc.sync.dma_start(out=_mo(t), in_=_mo(ap[None, :]))
            return t
        bv_r = load_row(bvd)

        # LN scale/shift per-partition columns [P, DO] (only if non-identity)
        def load_colT(ap, pspool):
            # [512] -> sbuf [4,128] -> PE transpose -> [128,4]
            nat = small.tile([DO, P], f32, tag="lncol_nat")
            nc.sync.dma_start(out=nat, in_=ap.rearrange("(a b) -> a b", b=P))
            ps = pspool.tile([P, DO], f32, tag="lncol_ps")
            nc.tensor.transpose(ps, nat, ident[:DO, :DO])
            t = consts.tile([P, DO], f32, tag=f"lncol_{ap.tensor.name}")
            nc.vector.tensor_copy(out=t, in_=ps)
            return t

        # mask bias column [P, nkb]: 0 where mask==1, -100 where mask==0
        with tc.tile_pool(name="ps_init", bufs=2, space="PSUM") as ps_init:
            mask_nat = small.tile([nkb, P], mybir.dt.int32, tag="mask_nat")
            nc.sync.dma_start(out=mask_nat,
                              in_=Md.rearrange("(a b) -> a b", b=P))
            mask_f = small.tile([nkb, P], f32, tag="mask_f")
            nc.vector.tensor_copy(out=mask_f, in_=mask_nat)  # int -> float cast
            mask_ps = ps_init.tile([P, nkb], f32, tag="mask_ps")
            nc.tensor.transpose(mask_ps, mask_f, ident[:nkb, :nkb])
            mb = consts.tile([P, nkb], f32)
            # (m - 1) * 100 : 1 -> 0, 0 -> -100
            nc.vector.tensor_scalar(out=mb, in0=mask_ps,
                                    scalar1=-1.0, scalar2=100.0,
                                    op0=OP.add, op1=OP.mult)

            g0T = load_colT(g0d, ps_init) if apply_g0b0 else None
            b0T = load_colT(b0d, ps_init) if apply_g0b0 else None
            g1T = load_colT(g1d, ps_init) if apply_g1b1 else None
            b1T = load_colT(b1d, ps_init) if apply_g1b1 else None
            bqT = load_colT(bqd, ps_init)
            bkT = load_colT(bkd, ps_init)
            boT = load_colT(bod, ps_init)


        # ---------- phase B: projections ----------
        qT = bigs.tile([P, DO, NQ], f32, tag="buf_qproj")
        qTb = bigs.tile([P, DO, NQ], BF16, tag="buf_qproj_bf")
        kT = bigs.tile([P, DO, NKC], BF16, tag="buf_kproj")
        vA = bigs.tile([P, nkb, H * (HD + 1)], BF16, tag="buf_v")

        with tc.tile_pool(name="ps_proj", bufs=8, space="PSUM") as ps_proj:
            # qT[dv, nq] = Wq^T @ Q^T + bq x 1 ; same for kT
            qchunks = [(qc * 512, 512) for qc in range(QC)]
            for WT, XT_src, dstT, dstT2, bT, ch in (
                    (Wq, QT, qT, qTb, bqT, qchunks),
                    (Wk, KT, kT, None, bkT, kchunks)):
                for do in range(DO):
                    for off, w in ch:
                        ps = ps_proj.tile([P, 512], f32, tag="projps")
                        for ko in range(DO):
                            nc.tensor.matmul(
                                ps[:, :w],
                                lhsT=_mm(WT[:, ko, do * P:(do + 1) * P]),
                                rhs=_mm(XT_src[:, ko, off:off + w]),
                                start=(ko == 0), stop=(ko == DO - 1))
                        # bias folded into the psum->sbuf copy (per-partition).
                        # kT is stored bf16 (scores-matmul operand); qT keeps
                        # an fp32 copy for the residual plus a bf16 copy.
                        dsl = dstT[:, do, off:off + w]
                        nc.vector.tensor_scalar_add(
                            out=dsl if dstT.dtype == BF16 else _mo(dsl),
                            in0=ps[:, :w], scalar1=bT[:, do:do + 1])
                        if dstT2 is not None:
                            nc.vector.tensor_scalar_add(
                                out=dstT2[:, do, off:off + w],
                                in0=ps[:, :w], scalar1=bT[:, do:do + 1])
            # v[nk, dv] = K @ Wv + 1 x bv, stored augmented per head:
            # vA[:, no, h*65 : h*65+64] = v columns of head h, vA[.., h*65+64] = 1.
            # The ones column makes the U matmul also produce the softmax
            # denominator in psum row 64 (fp32r matmuls must write base
            # partition 0, so the denominator must ride along, not col-pack).
            for no in range(nkb):
                ps = ps_proj.tile([P, 512], f32, tag="projps")
                for ko in range(DO):
                    nc.tensor.matmul(
                        ps,
                        lhsT=_mm(KT[:, ko, no * P:(no + 1) * P]),
                        rhs=_mm(Wv[:, ko, :]),
                        start=(ko == 0), stop=False)
                nc.tensor.matmul(
                    ps, lhsT=_mm(ones_row[:, :P]), rhs=_mm(bv_r),
                    start=False, stop=True)
                va = vA[:, no, :].rearrange("p (h e) -> p h e", e=HD + 1)
                nc.scalar.copy(out=va[:, :, :HD],
                               in_=ps.rearrange("p (h e) -> p h e", e=HD))
                nc.vector.tensor_copy(
                    out=va[:, :, HD:HD + 1],
                    in_=ones_stage[:, :H].rearrange("p (a b) -> p a b", b=1))

        # ---------- phase C: attention (head pairs) ----------
        # ZT = qT + attn @ v   (unnormalized accumulate, then divide by rowsum)
        ZT = bigs.tile([P, DO, NQ], f32, tag="buf_zt")
        SCALE = 1.0 / np.sqrt(np.float32(D))

        # fp32r matmuls may only write PSUM at base partition 0, so each
        # head accumulates U (attn@v) in rows 0:64 of its own psum tile; the
        # augmented ones-column of vA makes row 64 the softmax denominator.
        # 1/den is broadcast back over 64 partitions by a small SBUF->SBUF
        # DMA (the only cross-partition mover), and odd heads' results are
        # DMA-shifted into partitions 64:128 of ZT.
        with tc.tile_pool(name="ps_att", bufs=2, space="PSUM") as ps_att, \
             tc.tile_pool(name="ps_sc", bufs=2, space="PSUM") as ps_sc, \
             tc.tile_pool(name="pt_pool", bufs=4) as pt_pool, \
             tc.tile_pool(name="den_pool", bufs=3) as den_pool, \
             tc.tile_pool(name="den_dram", bufs=3, space="DRAM") as den_dram:
            for h in range(H):
                hp, hh = divmod(h, 2)
                r0 = HD * hh
                Ups = ps_att.tile([HD + 1, NQ], f32, tag="u", name=f"U{h}")
                for kb in range(nkb):
                    sc = ps_sc.tile([P, NQ], f32, tag="scores",
                                    name=f"S{h}_{kb}")
                    for qc in range(QC):
                        nc.tensor.matmul(
                            sc[:, qc * 512:(qc + 1) * 512],
                            lhsT=kT[r0:r0 + HD, hp, kb * P:(kb + 1) * P],
                            rhs=qTb[r0:r0 + HD, hp, qc * 512:(qc + 1) * 512],
                            start=True, stop=True)
                    # exp((s * scale) + maskbias) ; PSUM -> SBUF (bf16)
                    pt = pt_pool.tile([P, NQ], BF16, tag="pt")
                    nc.scalar.activation(pt, sc, AF.Exp,
                                         bias=mb[:, kb:kb + 1], scale=SCALE)
                    # [U ; den] += [v_h | 1]^T @ p
                    for qc in range(QC):
                        nc.tensor.matmul(
                            Ups[:, qc * 512:(qc + 1) * 512],
                            lhsT=vA[:, kb, h * (HD + 1):(h + 1) * (HD + 1)],
                            rhs=pt[:, qc * 512:(qc + 1) * 512],
                            start=(kb == 0), stop=(kb == nkb - 1))
                # head output: U / den (+ qT residual) into ZT rows r0:r0+64
                rrow = den_pool.tile([HD + 1, NQ], f32, tag="rrow")
                nc.vector.reciprocal(out=rrow[HD:HD + 1, :],
                                     in_=Ups[HD:HD + 1, :])
                # cross-partition broadcast of 1/den: SBUF row -> DRAM -> all
                # 64 partitions (DRAM APs may have stride-0 partition dims)
                dscratch = den_dram.tile([1, NQ], f32, tag="dd")
                nc.sync.dma_start(out=dscratch, in_=rrow[HD:HD + 1, :])
                rec = den_pool.tile([HD, NQ], f32, tag="rec")
                nc.sync.dma_start(out=rec, in_=dscratch.to_broadcast((HD, NQ)))
                if hh == 0:
                    nc.vector.tensor_mul(out=_mo(ZT[:HD, hp, :]),
                                         in0=Ups[:HD, :], in1=rec)
                    nc.gpsimd.tensor_add(out=_mo(ZT[:HD, hp, :]),
                                         in0=ZT[:HD, hp, :],
                                         in1=qT[:HD, hp, :])
                else:
                    tmp = den_pool.tile([HD, NQ], f32, tag="tmp")
                    nc.vector.tensor_mul(out=_mo(tmp), in0=Ups[:HD, :], in1=rec)
                    nc.sync.dma_start(out=_mo(ZT[HD:P, hp, :]), in_=_mo(tmp))
                    nc.gpsimd.tensor_add(out=_mo(ZT[HD:P, hp, :]),
                                         in0=ZT[HD:P, hp, :],
                                         in1=qT[HD:P, hp, :])

        # ---------- layernorm helper (d-major) ----------
        def layernorm(srcT, dstT, gT, bT, ps_pool, sq_pool, st_pool,
                      round_out=False, out_dma=None):
            """dstT = LN(srcT) over the d axis (partitions+groups)."""
            for qc in range(QC):
                s1 = ps_pool.tile([P, 512], f32, tag="s1")
                s2 = ps_pool.tile([P, 512], f32, tag="s2")
                for do in range(DO):
                    nc.tensor.matmul(
                        s1, lhsT=_mm(ones_pp), rhs=_mm(srcT[:, do, qc * 512:(qc + 1) * 512]),
                        start=(do == 0), stop=(do == DO - 1))
                for do in range(DO):
                    sq = sq_pool.tile([P, 512], f32, tag="sq")
                    nc.vector.tensor_mul(out=_mo(sq),
                                         in0=srcT[:, do, qc * 512:(qc + 1) * 512],
                                         in1=srcT[:, do, qc * 512:(qc + 1) * 512])
                    nc.tensor.matmul(s2, lhsT=_mm(ones_pp), rhs=_mm(sq),
                                     start=(do == 0), stop=(do == DO - 1))
                mu = st_pool.tile([P, 512], f32, tag="mu")
                nc.vector.tensor_scalar_mul(out=mu, in0=s1, scalar1=1.0 / D)
                ex2 = st_pool.tile([P, 512], f32, tag="ex2")
                nc.vector.tensor_scalar_mul(out=ex2, in0=s2, scalar1=1.0 / D)
                musq = st_pool.tile([P, 512], f32, tag="musq")
                nc.scalar.square(out=musq, in_=mu)
                var = st_pool.tile([P, 512], f32, tag="var")
                nc.vector.tensor_sub(out=var, in0=ex2, in1=musq)
                sd = st_pool.tile([P, 512], f32, tag="sd")
                nc.scalar.activation(sd, var, AF.Sqrt, bias=epsT)
                rstd = st_pool.tile([P, 512], f32, tag="rstd")
                nc.vector.reciprocal(out=rstd, in_=sd)
                for do in range(DO):
                    dslice = dstT[:, do, qc * 512:(qc + 1) * 512]
                    sslice = srcT[:, do, qc * 512:(qc + 1) * 512]
                    ro = _mo if round_out else (lambda x: x)
                    nc.gpsimd.tensor_sub(out=ro(dslice), in0=sslice, in1=mu)
                    nc.vector.tensor_mul(out=ro(dslice), in0=dslice, in1=rstd)
                    if gT is not None:
                        nc.vector.tensor_scalar(
                            out=ro(dslice), in0=dslice,
                            scalar1=gT[:, do:do + 1], scalar2=bT[:, do:do + 1],
                            op0=OP.mult, op1=OP.add)
                    if out_dma is not None:
                        nc.sync.dma_start(out=out_dma[:, do, qc * 512:(qc + 1) * 512],
                                          in_=dslice)

        # ---------- phase D: LN0 ----------
        XT = bigs.tile([P, DO, NQ], f32, tag="buf_kt")  # reuse KT slot
        with tc.tile_pool(name="ps_ln0", bufs=2, space="PSUM") as ps_ln0, \
             tc.tile_pool(name="sq0", bufs=4) as sq0, \
             tc.tile_pool(name="st0", bufs=2) as st0:
            layernorm(ZT, XT, g0T, b0T, ps_ln0, sq0, st0, round_out=True)

        # ---------- phase E: FC + relu + residual ----------
        Z2 = bigs.tile([P, DO, NQ], f32, tag="buf_qt")  # reuse QT slot
        with tc.tile_pool(name="ps_fc", bufs=8, space="PSUM") as ps_fc, \
             tc.tile_pool(name="fc_sb", bufs=3) as fc_sb:
            for do in range(DO):
                for qc in range(QC):
                    ps = ps_fc.tile([P, 512], f32, tag="fcps")
                    for ko in range(DO):
                        nc.tensor.matmul(
                            ps,
                            lhsT=_mm(Wo[:, ko, do * P:(do + 1) * P]),
                            rhs=_mm(XT[:, ko, qc * 512:(qc + 1) * 512]),
                            start=(ko == 0), stop=(ko == DO - 1))
                    rel = fc_sb.tile([P, 512], f32, tag="rel")
                    nc.scalar.activation(rel, ps, AF.Relu,
                                         bias=boT[:, do:do + 1])
                    nc.vector.tensor_add(
                        out=_mo(Z2[:, do, qc * 512:(qc + 1) * 512]),
                        in0=rel,
                        in1=XT[:, do, qc * 512:(qc + 1) * 512])

        # ---------- phase F: LN1 -> output ----------
        OT = bigs.tile([P, DO, NQ], f32, tag="buf_zt")  # reuse ZT slot
        with tc.tile_pool(name="ps_ln1", bufs=2, space="PSUM") as ps_ln1, \
             tc.tile_pool(name="sq1", bufs=4) as sq1, \
             tc.tile_pool(name="st1", bufs=2) as st1:
            layernorm(Z2, OT, g1T, b1T, ps_ln1, sq1, st1,
                      out_dma=OTd.rearrange("(o p) q -> p o q", p=P))


# ------------------------------------------------------------------
# host-side entry point
# ------------------------------------------------------------------
_CACHE = {}


def _get_program(repeat, apply_g0b0, apply_g1b1, nkb=NO):
    key = (repeat, apply_g0b0, apply_g1b1, nkb)
    if key not in _CACHE:
        _CACHE[key] = build_program(repeat, apply_g0b0, apply_g1b1, nkb)
    return _CACHE[key]


def compact_keys(K_b, mask_b, nkb):
    """Move unmasked keys to the front (order-preserving) and truncate to
    nkb*128 rows.  Softmax over keys is permutation-invariant and fully
    masked keys contribute exactly zero, so this is output-preserving as
    long as all unmasked keys survive the truncation."""
    nkc = nkb * P
    order = np.argsort(mask_b == 0, kind="stable")[:nkc]
    return (np.ascontiguousarray(K_b[order]),
            np.ascontiguousarray(mask_b[order]))


def pick_nkb(attention_mask):
    counts = (np.asarray(attention_mask) != 0).sum(axis=-1)
    return max(1, min(NO, int(-(-int(counts.max()) // P))))


def make_in_maps(inputs, nkb):
    shared = {k: np.asarray(inputs[k], np.float32)
              for k in ("Wq", "Wk", "Wv", "Wo", "bq", "bk", "bv", "bo",
                        "g0", "b0", "g1", "b1")}
    Q = np.asarray(inputs["Q"], np.float32)
    K = np.asarray(inputs["K"], np.float32)
    mask = np.asarray(inputs["attention_mask"], np.int32)
    in_maps = []
    for b in range(B):
        m = dict(shared)
        m["Q"] = np.ascontiguousarray(Q[b])
        Kc, mc = compact_keys(K[b], mask[b], nkb)
        m["K"] = Kc
        m["attention_mask"] = mc
        in_maps.append(m)
    return in_maps


def kernel(Q, K, attention_mask, Wq, bq, Wk, bk, Wv, bv, Wo, bo,
           g0, b0, g1, b1, _repeat=1):
    inputs = {
        "Q": Q, "K": K, "attention_mask": attention_mask,
        "Wq": Wq, "bq": bq, "Wk": Wk, "bk": bk, "Wv": Wv, "bv": bv,
        "Wo": Wo, "bo": bo, "g0": g0, "b0": b0, "g1": g1, "b1": b1,
    }
    apply_g0b0 = not (np.all(np.asarray(g0) == 1.0)
                      and np.all(np.asarray(b0) == 0.0))
    apply_g1b1 = not (np.all(np.asarray(g1) == 1.0)
                      and np.all(np.asarray(b1) == 0.0))
    nkb = pick_nkb(attention_mask)
    nc = _get_program(_repeat, apply_g0b0, apply_g1b1, nkb)
    in_maps = make_in_maps(inputs, nkb)

    res = bass_utils.run_bass_kernel_spmd(
        nc, in_maps, core_ids=list(range(N_CORES)), trace=False)

    out = np.empty((B, NQ, D), np.float32)
    for b in range(B):
        out[b] = res.results[b]["OT"].T
    return out



# revision 5
# speedup vs baseline: 1.0777x; 1.0777x over previous
"""Trainium2 Bass kernel for nn_MAB_72911364817388 (dense transformer block).

Reference computation (per batch element b):
    q = Q @ Wq + bq ; k = K @ Wk + bk ; v = K @ Wv + bv        (1024x512 @ 512x512)
    scores = einsum("qhd,khd->hqk", qh, kh) / sqrt(512)
    scores = where(mask==0, -1e4, scores); attn = softmax(scores, axis=k)
    oh = qh + attn @ vh ; O = LN0(oh) ; O = O + relu(O @ Wo + bo) ; O = LN1(O)

Strategy: pure data-parallel over batch B=8 -> one batch element per core,
no collectives.  All activations are d-major (feature dim on partitions) in
bf16; psum accumulation is fp32.

Host-side prep: Q, K and the four weight matrices are cast to bf16 (halves
DMA bytes; bf16 matmuls stream 1 col/cycle with cheap self-loading weights).
Unmasked keys are compacted to the front (softmax is permutation-invariant,
fully-masked keys contribute exactly zero), so attention processes
nkb*128 ~ 640 of 1024 keys.  Per-partition constants (bias columns, mask
bias) are packed host-side into one [128, n] fp32 array -> single DMA, no
on-device transposes.

Q^T / K^T are produced by XBAR DMA-transposes (bf16) straight from DRAM --
no PE transposes, no psum->sbuf copies.

Attention per head: lhsT = [v_h | ones*64] (128x128), so the U matmul
leaves attn@v in psum rows 0:64 and the softmax denominator replicated in
rows 64:128.  Epilogue: DVE reciprocal (psum->sbuf) + DVE multiply writing
the head's 64 rows of ZT directly (partition-base remap on write), + Pool
residual add of q.  Softmax is computed unnormalized: p = exp(s/sqrt(512) +
maskbias) with maskbias=-100 for masked keys (exp underflows to 0 in bf16).

LayerNorms: token-wise stats via ones-matmul on the PE (replicated across
partitions for free); finalize via fused scalar_tensor_tensor chains;
apply via broadcast tensor_tensor ops.  The "-mu*rstd" shift of LN0 is
never materialized: it is folded into the FC matmul as a rank-1 update
(w1 = colsum(Wo), host-precomputed) and the uniform per-token shift left
in the residual stream is absorbed by LN1 (layernorm is invariant to
per-token uniform shifts).
"""

import numpy as np
import ml_dtypes

import concourse.bass as bass
import concourse.mybir as mybir
import concourse.tile as tile
from concourse import bacc, bass_utils

# Problem shapes (hardcoded per contract).
B = 8
NQ = NK = 1024
D = 512  # DQ = DK = DV
H = 8
HD = 64
P = 128
EPS = 1e-5
N_CORES = 8

DO = D // P  # 4   d-major partition groups
NO = NQ // P  # 8
QC = NQ // 512  # 2 free-dim chunks of 512

F32 = mybir.dt.float32
BF16 = mybir.dt.bfloat16
NP_BF16 = ml_dtypes.bfloat16

AF = mybir.ActivationFunctionType
OP = mybir.AluOpType

SCALE = float(1.0 / np.sqrt(np.float32(D)))


def build_program(repeat: int = 1, apply_g0b0: bool = False,
                  apply_g1b1: bool = False, nkb: int = NO):
    nc = bacc.Bacc("TRN2", target_bir_lowering=False, debug=False,
                   num_devices=N_CORES)

    NKC = nkb * P
    ncol = 12 + nkb + (8 if apply_g0b0 else 0) + (8 if apply_g1b1 else 0)

    Qd = nc.dram_tensor("Q", [NQ, D], BF16, kind="ExternalInput").ap()
    Kd = nc.dram_tensor("K", [NKC, D], BF16, kind="ExternalInput").ap()
    Wqd = nc.dram_tensor("Wq", [D, D], BF16, kind="ExternalInput").ap()
    Wkd = nc.dram_tensor("Wk", [D, D], BF16, kind="ExternalInput").ap()
    Wvd = nc.dram_tensor("Wv", [D, D], BF16, kind="ExternalInput").ap()
    Wod = nc.dram_tensor("Wo", [D, D], BF16, kind="ExternalInput").ap()
    bvd = nc.dram_tensor("bvb", [D], BF16, kind="ExternalInput").ap()
    w1d = nc.dram_tensor("w1", [D], BF16, kind="ExternalInput").ap()
    COLSd = nc.dram_tensor("COLS", [P, ncol], F32, kind="ExternalInput").ap()
    # Output is O^T (d-major) in bf16; the host casts + transposes back.
    OTd = nc.dram_tensor("OT", [D, NQ], BF16, kind="ExternalOutput").ap()

    with tile.TileContext(nc) as tc:
        def body():
            _build_body(nc, tc, Qd, Kd, Wqd, Wkd, Wvd, Wod, bvd, w1d,
                        COLSd, OTd, apply_g0b0, apply_g1b1, nkb)

        if repeat == 1:
            body()
        else:
            with tc.For_i(0, repeat, 1,
                          hint_engines=(mybir.EngineType.PE,
                                        mybir.EngineType.Activation,
                                        mybir.EngineType.DVE,
                                        mybir.EngineType.SP,
                                        mybir.EngineType.Pool)):
                body()

    nc.compile()
    return nc


def _build_body(nc, tc, Qd, Kd, Wqd, Wkd, Wvd, Wod, bvd, w1d,
                COLSd, OTd, apply_g0b0, apply_g1b1, nkb):
    NKC = nkb * P
    ncol = 12 + nkb + (8 if apply_g0b0 else 0) + (8 if apply_g1b1 else 0)
    kchunks = []
    off = 0
    while off < NKC:
        w = min(512, NKC - off)
        kchunks.append((off, w))
        off += w

    import contextlib
    ctx = contextlib.ExitStack()
    with ctx:
        consts = ctx.enter_context(tc.tile_pool(name="consts", bufs=1))
        bigs = ctx.enter_context(tc.tile_pool(name="bigs", bufs=1))
        ptp = ctx.enter_context(tc.tile_pool(name="ptp", bufs=3))
        recp = ctx.enter_context(tc.tile_pool(name="recp", bufs=2))
        stp = ctx.enter_context(tc.tile_pool(name="stp", bufs=3))
        abp = ctx.enter_context(tc.tile_pool(name="abp", bufs=4))
        ps_sc = ctx.enter_context(
            tc.tile_pool(name="ps_sc", bufs=2, space="PSUM"))
        ps_u = ctx.enter_context(
            tc.tile_pool(name="ps_u", bufs=1, space="PSUM"))
        ps_s = ctx.enter_context(
            tc.tile_pool(name="ps_s", bufs=2, space="PSUM"))

        # ---------------- constants / inputs ----------------
        ones_bf = consts.tile([P, 512], BF16)
        nc.vector.memset(ones_bf, 1.0)
        epsT = consts.tile([P, 1], F32)
        nc.vector.memset(epsT, EPS)

        COLS = consts.tile([P, ncol], F32)
        nc.sync.dma_start(out=COLS, in_=COLSd)
        bqT = COLS[:, 0:4]
        bkT = COLS[:, 4:8]
        boT = COLS[:, 8:12]
        mbT = COLS[:, 12:12 + nkb]
        co = 12 + nkb
        if apply_g0b0:
            g0T, b0T = COLS[:, co:co + 4], COLS[:, co + 4:co + 8]
            co += 8
        if apply_g1b1:
            g1T, b1T = COLS[:, co:co + 4], COLS[:, co + 4:co + 8]

        KT = bigs.tile([P, DO, NKC], BF16, tag="kt")
        for o in range(DO):
            nc.sync.dma_start_transpose(
                KT[:, o, :], Kd[:, o * P:(o + 1) * P])
        Wv = consts.tile([P, DO, D], BF16)
        for o in range(DO):
            nc.sync.dma_start(
                out=Wv[:, o, :],
                in_=Wvd.rearrange("(o p) n -> p o n", p=P)[:, o, :])
        bv_r = consts.tile([1, D], BF16)
        nc.sync.dma_start(out=bv_r, in_=bvd[None, :])

        QT = bigs.tile([P, DO, NQ], BF16, tag="qt")
        for o in range(DO):
            for h2 in range(2):
                nc.sync.dma_start_transpose(
                    QT[:, o, h2 * 512:(h2 + 1) * 512],
                    Qd[h2 * 512:(h2 + 1) * 512, o * P:(o + 1) * P])
        Wq = consts.tile([P, DO, D], BF16)
        Wk = consts.tile([P, DO, D], BF16)
        Wo = consts.tile([P, DO, D], BF16)
        for o in range(DO):
            nc.sync.dma_start(
                out=Wq[:, o, :],
                in_=Wqd.rearrange("(o p) n -> p o n", p=P)[:, o, :])
        for o in range(DO):
            nc.sync.dma_start(
                out=Wk[:, o, :],
                in_=Wkd.rearrange("(o p) n -> p o n", p=P)[:, o, :])
        for o in range(DO):
            nc.sync.dma_start(
                out=Wo[:, o, :],
                in_=Wod.rearrange("(o p) n -> p o n", p=P)[:, o, :])
        w1_r = consts.tile([1, D], BF16)
        nc.sync.dma_start(out=w1_r, in_=w1d[None, :])

        # ---------------- v projection ----------------
        # v[keys, dv] = K @ Wv + 1 (x) bv, stored per (kb, head) as
        # [v_h (64 cols) | ones (64 cols)] so the U matmul produces the
        # softmax denominator replicated over psum rows 64:128.
        vA = bigs.tile([P, nkb, H, P], BF16, tag="va")
        for kb in range(nkb):
            ps = ps_s.tile([P, 512], F32, tag="pp")
            for ko in range(DO):
                nc.tensor.matmul(ps, lhsT=KT[:, ko, kb * P:(kb + 1) * P],
                                 rhs=Wv[:, ko, :],
                                 start=(ko == 0), stop=False)
            nc.tensor.matmul(ps, lhsT=ones_bf[:1, :P], rhs=bv_r,
                             start=False, stop=True)
            nc.scalar.copy(out=vA[:, kb, :, 0:HD],
                           in_=ps.rearrange("p (h e) -> p h e", e=HD))
            nc.gpsimd.tensor_copy(
                out=vA[:, kb, :, HD:P],
                in_=ones_bf.rearrange("p (h e) -> p h e", e=HD))

        # ---------------- projections + attention (interleaved) ----------
        qTb = bigs.tile([P, DO, NQ], BF16, tag="qproj")
        kTp = bigs.tile([P, DO, NKC], BF16, tag="kproj")
        ZT = bigs.tile([P, DO, NQ], BF16, tag="zt")

        def proj(g):
            for qc in range(QC):
                ps = ps_s.tile([P, 512], F32, tag="pp")
                for ko in range(DO):
                    nc.tensor.matmul(
                        ps, lhsT=Wq[:, ko, g * P:(g + 1) * P],
                        rhs=QT[:, ko, qc * 512:(qc + 1) * 512],
                        start=(ko == 0), stop=(ko == DO - 1))
                nc.vector.tensor_scalar_add(
                    out=qTb[:, g, qc * 512:(qc + 1) * 512],
                    in0=ps, scalar1=bqT[:, g:g + 1])
            for off, w in kchunks:
                ps = ps_s.tile([P, 512], F32, tag="pp")
                for ko in range(DO):
                    nc.tensor.matmul(
                        ps[:, :w], lhsT=Wk[:, ko, g * P:(g + 1) * P],
                        rhs=KT[:, ko, off:off + w],
                        start=(ko == 0), stop=(ko == DO - 1))
                nc.vector.tensor_scalar_add(
                    out=kTp[:, g, off:off + w],
                    in0=ps[:, :w], scalar1=bkT[:, g:g + 1])

        head_state = {}

        def head_mm(h):
            g, hh = divmod(h, 2)
            r0 = HD * hh
            ups = ps_u.tile([P, NQ], F32, tag="u", name=f"U{h}")
            for kb in range(nkb):
                sc = ps_sc.tile([P, NQ], F32, tag="sc", name=f"S{h}_{kb}")
                for qc in range(QC):
                    nc.tensor.matmul(
                        sc[:, qc * 512:(qc + 1) * 512],
                        lhsT=kTp[r0:r0 + HD, g, kb * P:(kb + 1) * P],
                        rhs=qTb[r0:r0 + HD, g, qc * 512:(qc + 1) * 512],
                        start=True, stop=True)
                pt = ptp.tile([P, NQ], BF16, tag="pt")
                nc.scalar.activation(pt, sc, AF.Exp,
                                     bias=mbT[:, kb:kb + 1], scale=SCALE)
                for qc in range(QC):
                    nc.tensor.matmul(
                        ups[:, qc * 512:(qc + 1) * 512],
                        lhsT=vA[:, kb, h, :],
                        rhs=pt[:, qc * 512:(qc + 1) * 512],
                        start=(kb == 0), stop=(kb == nkb - 1))
            head_state[h] = ups

        def head_epi(h):
            g, hh = divmod(h, 2)
            r0 = HD * hh
            ups = head_state.pop(h)
            rec = recp.tile([HD, NQ], F32, tag="rec")
            nc.vector.reciprocal(out=rec, in_=ups[HD:P, :])
            dst = ZT[r0:r0 + HD, g, :]
            nc.vector.tensor_tensor(out=dst, in0=ups[0:HD, :], in1=rec,
                                    op=OP.mult)
            nc.gpsimd.tensor_tensor(out=dst, in0=dst,
                                    in1=qTb[r0:r0 + HD, g, :], op=OP.add)

        proj(0)
        for g in range(DO):
            head_mm(2 * g)
            if g < DO - 1:
                proj(g + 1)
            head_epi(2 * g)
            head_mm(2 * g + 1)
            head_epi(2 * g + 1)

        # ---------------- layernorm helper ----------------
        def layernorm(src, c_is_mu_only):
            """Returns (A, C): A = rstd (bf16 [P, NQ], replicated), and
            C = -mu (c_is_mu_only) or -mu*rstd, both bf16 replicated.
            src: bf16 [P, DO, NQ] tile."""
            sq = bigs.tile([P, DO, NQ], BF16, tag="sq")
            srcv = src.rearrange("p a t -> p (a t)")
            nc.vector.tensor_tensor(
                out=sq.rearrange("p a t -> p (a t)"),
                in0=srcv, in1=srcv, op=OP.mult)
            s1 = ps_sc.tile([P, NQ], F32, tag="sc", name="s1")
            s2 = ps_sc.tile([P, NQ], F32, tag="sc", name="s2")
            for qc in range(QC):
                for ko in range(DO):
                    nc.tensor.matmul(
                        s1[:, qc * 512:(qc + 1) * 512],
                        lhsT=ones_bf[:, :P],
                        rhs=src[:, ko, qc * 512:(qc + 1) * 512],
                        start=(ko == 0), stop=(ko == DO - 1))
                for ko in range(DO):
                    nc.tensor.matmul(
                        s2[:, qc * 512:(qc + 1) * 512],
                        lhsT=ones_bf[:, :P],
                        rhs=sq[:, ko, qc * 512:(qc + 1) * 512],
                        start=(ko == 0), stop=(ko == DO - 1))
            t1 = stp.tile([P, NQ], F32, tag="t1")
            nc.scalar.activation(t1, s1, AF.Square, scale=1.0 / D)
            t2 = stp.tile([P, NQ], F32, tag="t2")
            nc.vector.scalar_tensor_tensor(
                out=t2, in0=s2, scalar=1.0 / D, in1=t1,
                op0=OP.mult, op1=OP.subtract)
            t3 = stp.tile([P, NQ], F32, tag="t3")
            nc.scalar.activation(t3, t2, AF.Sqrt, bias=epsT)
            A = abp.tile([P, NQ], BF16, tag="A")
            with nc.allow_low_precision("bf16 rstd is plenty for 2e-2"):
                nc.vector.reciprocal(out=A, in_=t3)
            C = abp.tile([P, NQ], BF16, tag="C")
            if c_is_mu_only:
                nc.vector.tensor_scalar_mul(out=C, in0=s1, scalar1=-1.0 / D)
            else:
                nc.vector.scalar_tensor_tensor(
                    out=C, in0=s1, scalar=-1.0 / D, in1=A,
                    op0=OP.mult, op1=OP.mult)
            return A, C

        def bcast(t):
            return t.rearrange("p (a t) -> p a t", a=1).to_broadcast(
                (P, DO, NQ))

        # ---------------- LN0 ----------------
        A0, C0 = layernorm(ZT, c_is_mu_only=False)
        XT = bigs.tile([P, DO, NQ], BF16, tag="xt")
        nc.vector.tensor_tensor(out=XT, in0=ZT.rearrange("p a t -> p a t"),
                                in1=bcast(A0), op=OP.mult)
        if apply_g0b0:
            # materialize the full LN0 output (no rank-1 folding)
            nc.vector.tensor_tensor(out=XT, in0=XT, in1=bcast(C0), op=OP.add)
            for g in range(DO):
                nc.vector.tensor_scalar(
                    out=XT[:, g, :], in0=XT[:, g, :],
                    scalar1=g0T[:, g:g + 1], scalar2=b0T[:, g:g + 1],
                    op0=OP.mult, op1=OP.add)

        # ---------------- FC + relu + residual ----------------
        RT = bigs.tile([P, DO, NQ], BF16, tag="rt")
        for g in range(DO):
            for qc in range(QC):
                ps = ps_s.tile([P, 512], F32, tag="pp")
                for ko in range(DO):
                    nc.tensor.matmul(
                        ps, lhsT=Wo[:, ko, g * P:(g + 1) * P],
                        rhs=XT[:, ko, qc * 512:(qc + 1) * 512],
                        start=(ko == 0), stop=apply_g0b0 and ko == DO - 1)
                if not apply_g0b0:
                    # rank-1: + w1 (x) C0   (the -mu*rstd shift of LN0)
                    nc.tensor.matmul(
                        ps, lhsT=w1_r[:1, g * P:(g + 1) * P],
                        rhs=C0[0:1, qc * 512:(qc + 1) * 512],
                        start=False, stop=True)
                nc.vector.tensor_scalar(
                    out=RT[:, g, qc * 512:(qc + 1) * 512], in0=ps,
                    scalar1=boT[:, g:g + 1], scalar2=0.0,
                    op0=OP.add, op1=OP.max)
        O2 = bigs.tile([P, DO, NQ], BF16, tag="o2")
        nc.vector.tensor_tensor(out=O2, in0=XT, in1=RT, op=OP.add)

        # ---------------- LN1 -> output ----------------
        # LN1 absorbs the uniform per-token shift C0 left out of the
        # residual stream (layernorm is shift-invariant per token).
        A1, C1 = layernorm(O2, c_is_mu_only=True)
        TA = bigs.tile([P, DO, NQ], BF16, tag="ta")
        nc.vector.tensor_tensor(out=TA, in0=O2, in1=bcast(C1), op=OP.add)
        OTb = bigs.tile([P, DO, NQ], BF16, tag="otb")
        nc.vector.tensor_tensor(out=OTb, in0=TA, in1=bcast(A1), op=OP.mult)
        if apply_g1b1:
            for g in range(DO):
                nc.vector.tensor_scalar(
                    out=OTb[:, g, :], in0=OTb[:, g, :],
                    scalar1=g1T[:, g:g + 1], scalar2=b1T[:, g:g + 1],
                    op0=OP.mult, op1=OP.add)
        OTv = OTd.rearrange("(g p) t -> p g t", p=P)
        for g in range(DO):
            nc.sync.dma_start(out=OTv[:, g, :], in_=OTb[:, g, :])


# ------------------------------------------------------------------
# host-side entry point
# ------------------------------------------------------------------
_CACHE = {}


def _get_program(repeat, apply_g0b0, apply_g1b1, nkb=NO):
    key = (repeat, apply_g0b0, apply_g1b1, nkb)
    if key not in _CACHE:
        _CACHE[key] = build_program(repeat, apply_g0b0, apply_g1b1, nkb)
    return _CACHE[key]


def compact_keys(K_b, mask_b, nkb):
    """Move unmasked keys to the front (order-preserving) and truncate to
    nkb*128 rows.  Softmax over keys is permutation-invariant and fully
    masked keys contribute exactly zero, so this is output-preserving as
    long as all unmasked keys survive the truncation."""
    nkc = nkb * P
    order = np.argsort(mask_b == 0, kind="stable")[:nkc]
    return (np.ascontiguousarray(K_b[order]),
            np.ascontiguousarray(mask_b[order]))


def pick_nkb(attention_mask):
    counts = (np.asarray(attention_mask) != 0).sum(axis=-1)
    return max(1, min(NO, int(-(-int(counts.max()) // P))))


def make_in_maps(inputs, nkb):
    f32 = np.float32
    Wq = np.asarray(inputs["Wq"], f32)
    Wk = np.asarray(inputs["Wk"], f32)
    Wv = np.asarray(inputs["Wv"], f32)
    Wo = np.asarray(inputs["Wo"], f32)
    g0 = np.asarray(inputs["g0"], f32)
    b0 = np.asarray(inputs["b0"], f32)
    g1 = np.asarray(inputs["g1"], f32)
    b1 = np.asarray(inputs["b1"], f32)
    apply_g0b0 = not (np.all(g0 == 1.0) and np.all(b0 == 0.0))
    apply_g1b1 = not (np.all(g1 == 1.0) and np.all(b1 == 0.0))

    # w1 is only consumed by the rank-1 C0-fold, which is disabled when
    # apply_g0b0 (the full LN0 output is materialized instead).
    w1 = Wo.sum(axis=0)
    bo = np.asarray(inputs["bo"], f32)

    shared = {
        "Wq": Wq.astype(NP_BF16), "Wk": Wk.astype(NP_BF16),
        "Wv": Wv.astype(NP_BF16), "Wo": Wo.astype(NP_BF16),
        "bvb": np.asarray(inputs["bv"], f32).astype(NP_BF16),
        "w1": w1.astype(NP_BF16),
    }

    def cols_for(mask_b):
        mb = np.where(mask_b != 0, 0.0, -100.0).astype(f32)
        parts = [np.asarray(inputs["bq"], f32).reshape(4, P).T,
                 np.asarray(inputs["bk"], f32).reshape(4, P).T,
                 bo.reshape(4, P).T,
                 mb.reshape(nkb, P).T]
        if apply_g0b0:
            parts += [g0.reshape(4, P).T, b0.reshape(4, P).T]
        if apply_g1b1:
            parts += [g1.reshape(4, P).T, b1.reshape(4, P).T]
        return np.ascontiguousarray(np.concatenate(parts, axis=1))

    Q = np.asarray(inputs["Q"], f32)
    K = np.asarray(inputs["K"], f32)
    mask = np.asarray(inputs["attention_mask"], np.int32)
    in_maps = []
    for b in range(B):
        m = dict(shared)
        Kc, mc = compact_keys(K[b], mask[b], nkb)
        m["Q"] = np.ascontiguousarray(Q[b]).astype(NP_BF16)
        m["K"] = Kc.astype(NP_BF16)
        m["COLS"] = cols_for(mc)
        in_maps.append(m)
    return in_maps


def kernel(Q, K, attention_mask, Wq, bq, Wk, bk, Wv, bv, Wo, bo,
           g0, b0, g1, b1, _repeat=1):
    inputs = {
        "Q": Q, "K": K, "attention_mask": attention_mask,
        "Wq": Wq, "bq": bq, "Wk": Wk, "bk": bk, "Wv": Wv, "bv": bv,
        "Wo": Wo, "bo": bo, "g0": g0, "b0": b0, "g1": g1, "b1": b1,
    }
    apply_g0b0 = not (np.all(np.asarray(g0) == 1.0)
                      and np.all(np.asarray(b0) == 0.0))
    apply_g1b1 = not (np.all(np.asarray(g1) == 1.0)
                      and np.all(np.asarray(b1) == 0.0))
    nkb = pick_nkb(attention_mask)
    nc = _get_program(_repeat, apply_g0b0, apply_g1b1, nkb)
    in_maps = make_in_maps(inputs, nkb)

    res = bass_utils.run_bass_kernel_spmd(
        nc, in_maps, core_ids=list(range(N_CORES)), trace=False)

    out = np.empty((B, NQ, D), np.float32)
    for b in range(B):
        out[b] = res.results[b]["OT"].astype(np.float32).T
    return out


# revision 7
# speedup vs baseline: 1.2204x; 1.1324x over previous
"""Trainium2 Bass kernel for nn_MAB_72911364817388 (dense transformer block).

Reference computation (per batch element b):
    q = Q @ Wq + bq ; k = K @ Wk + bk ; v = K @ Wv + bv        (1024x512 @ 512x512)
    scores = einsum("qhd,khd->hqk", qh, kh) / sqrt(512)
    scores = where(mask==0, -1e4, scores); attn = softmax(scores, axis=k)
    oh = qh + attn @ vh ; O = LN0(oh) ; O = O + relu(O @ Wo + bo) ; O = LN1(O)

Strategy: pure data-parallel over batch B=8 -> one batch element per core,
no collectives.  All activations are d-major (feature dim on partitions) in
bf16; psum accumulation is fp32.

Host-side prep: Q, K and the four weight matrices are cast to bf16 (halves
DMA bytes; bf16 matmuls stream 1 col/cycle with cheap self-loading weights).
Unmasked keys are compacted to the front (softmax is permutation-invariant,
fully-masked keys contribute exactly zero), so attention processes
nkb*128 ~ 640 of 1024 keys.  Per-partition constants (bias columns, mask
bias) are packed host-side into one [128, n] fp32 array -> single DMA, no
on-device transposes.

Q^T / K^T are produced by XBAR DMA-transposes (bf16) straight from DRAM --
no PE transposes, no psum->sbuf copies.

Attention per head: lhsT = [v_h | ones*64] (128x128), so the U matmul
leaves attn@v in psum rows 0:64 and the softmax denominator replicated in
rows 64:128.  Epilogue: DVE reciprocal (psum->sbuf) + DVE multiply writing
the head's 64 rows of ZT directly (partition-base remap on write), + Pool
residual add of q.  Softmax is computed unnormalized: p = exp(s/sqrt(512) +
maskbias) with maskbias=-100 for masked keys (exp underflows to 0 in bf16).

LayerNorms: token-wise stats via ones-matmul on the PE (replicated across
partitions for free); finalize via fused scalar_tensor_tensor chains;
apply via broadcast tensor_tensor ops.  The "-mu*rstd" shift of LN0 is
never materialized: it is folded into the FC matmul as a rank-1 update
(w1 = colsum(Wo), host-precomputed) and the uniform per-token shift left
in the residual stream is absorbed by LN1 (layernorm is invariant to
per-token uniform shifts).
"""

import numpy as np
import ml_dtypes

import concourse.bass as bass
import concourse.mybir as mybir
import concourse.tile as tile
from concourse import bacc, bass_utils

# Problem shapes (hardcoded per contract).
B = 8
NQ = NK = 1024
D = 512  # DQ = DK = DV
H = 8
HD = 64
P = 128
EPS = 1e-5
N_CORES = 8

DO = D // P  # 4   d-major partition groups
NO = NQ // P  # 8
QC = NQ // 512  # 2 free-dim chunks of 512

F32 = mybir.dt.float32
BF16 = mybir.dt.bfloat16
NP_BF16 = ml_dtypes.bfloat16

AF = mybir.ActivationFunctionType
OP = mybir.AluOpType

SCALE = float(1.0 / np.sqrt(np.float32(D)))


def build_program(repeat: int = 1, apply_g0b0: bool = False,
                  apply_g1b1: bool = False, nkb: int = NO):
    nc = bacc.Bacc("TRN2", target_bir_lowering=False, debug=False,
                   num_devices=N_CORES)

    NKC = nkb * P
    ncol = 12 + nkb + (8 if apply_g0b0 else 0) + (8 if apply_g1b1 else 0)

    Qd = nc.dram_tensor("Q", [NQ, D], BF16, kind="ExternalInput").ap()
    Kd = nc.dram_tensor("K", [NKC, D], BF16, kind="ExternalInput").ap()
    Wqd = nc.dram_tensor("Wq", [D, D], BF16, kind="ExternalInput").ap()
    Wkd = nc.dram_tensor("Wk", [D, D], BF16, kind="ExternalInput").ap()
    Wvd = nc.dram_tensor("Wv", [D, D], BF16, kind="ExternalInput").ap()
    Wod = nc.dram_tensor("Wo", [D, D], BF16, kind="ExternalInput").ap()
    bvd = nc.dram_tensor("bvb", [D], BF16, kind="ExternalInput").ap()
    w1d = nc.dram_tensor("w1", [D], BF16, kind="ExternalInput").ap()
    COLSd = nc.dram_tensor("COLS", [P, ncol], F32, kind="ExternalInput").ap()
    # Output is O^T (d-major) in bf16; the host casts + transposes back.
    OTd = nc.dram_tensor("OT", [D, NQ], BF16, kind="ExternalOutput").ap()

    with tile.TileContext(nc) as tc:
        def body():
            _build_body(nc, tc, Qd, Kd, Wqd, Wkd, Wvd, Wod, bvd, w1d,
                        COLSd, OTd, apply_g0b0, apply_g1b1, nkb)

        if repeat == 1:
            body()
        else:
            with tc.For_i(0, repeat, 1,
                          hint_engines=(mybir.EngineType.PE,
                                        mybir.EngineType.Activation,
                                        mybir.EngineType.DVE,
                                        mybir.EngineType.SP,
                                        mybir.EngineType.Pool)):
                body()

    nc.compile()
    return nc


def _build_body(nc, tc, Qd, Kd, Wqd, Wkd, Wvd, Wod, bvd, w1d,
                COLSd, OTd, apply_g0b0, apply_g1b1, nkb):
    NKC = nkb * P
    ncol = 12 + nkb + (8 if apply_g0b0 else 0) + (8 if apply_g1b1 else 0)
    kchunks = []
    off = 0
    while off < NKC:
        w = min(512, NKC - off)
        kchunks.append((off, w))
        off += w

    import contextlib
    ctx = contextlib.ExitStack()
    with ctx:
        consts = ctx.enter_context(tc.tile_pool(name="consts", bufs=1))
        bigs = ctx.enter_context(tc.tile_pool(name="bigs", bufs=1))
        ptp = ctx.enter_context(tc.tile_pool(name="ptp", bufs=4))
        recp = ctx.enter_context(tc.tile_pool(name="recp", bufs=2))
        stp = ctx.enter_context(tc.tile_pool(name="stp", bufs=3))
        abp = ctx.enter_context(tc.tile_pool(name="abp", bufs=4))
        # PSUM: two pools of [128,1024] tiles (2 banks each), 2 bufs each ->
        # all 8 banks.  ps_a: scores + projection/FC groups.  ps_b: per-head
        # U accumulators (live across the kb loop) + LN stats.
        ps_a = ctx.enter_context(
            tc.tile_pool(name="ps_a", bufs=2, space="PSUM"))
        ps_b = ctx.enter_context(
            tc.tile_pool(name="ps_b", bufs=2, space="PSUM"))

        # ---------------- constants / inputs ----------------
        ones_bf = consts.tile([P, 512], BF16)
        nc.vector.memset(ones_bf, 1.0)
        epsT = consts.tile([P, 1], F32)
        nc.vector.memset(epsT, EPS)

        COLS = consts.tile([P, ncol], F32)
        nc.sync.dma_start(out=COLS, in_=COLSd)
        bqT = COLS[:, 0:4]
        bkT = COLS[:, 4:8]
        boT = COLS[:, 8:12]
        mbT = COLS[:, 12:12 + nkb]
        co = 12 + nkb
        if apply_g0b0:
            g0T, b0T = COLS[:, co:co + 4], COLS[:, co + 4:co + 8]
            co += 8
        if apply_g1b1:
            g1T, b1T = COLS[:, co:co + 4], COLS[:, co + 4:co + 8]

        KT = bigs.tile([P, DO, NKC], BF16, tag="kt")
        for o in range(DO):
            nc.sync.dma_start_transpose(
                KT[:, o, :], Kd[:, o * P:(o + 1) * P])
        Wv = consts.tile([P, DO, D], BF16)
        for o in range(DO):
            nc.sync.dma_start(
                out=Wv[:, o, :],
                in_=Wvd.rearrange("(o p) n -> p o n", p=P)[:, o, :])
        bv_r = consts.tile([1, D], BF16)
        nc.sync.dma_start(out=bv_r, in_=bvd[None, :])

        QT = bigs.tile([P, DO, NQ], BF16, tag="qt")
        for o in range(DO):
            for h2 in range(2):
                nc.sync.dma_start_transpose(
                    QT[:, o, h2 * 512:(h2 + 1) * 512],
                    Qd[h2 * 512:(h2 + 1) * 512, o * P:(o + 1) * P])
        Wq = consts.tile([P, DO, D], BF16)
        Wk = consts.tile([P, DO, D], BF16)
        Wo = consts.tile([P, DO, D], BF16)
        for o in range(DO):
            nc.sync.dma_start(
                out=Wq[:, o, :],
                in_=Wqd.rearrange("(o p) n -> p o n", p=P)[:, o, :])
        for o in range(DO):
            nc.sync.dma_start(
                out=Wk[:, o, :],
                in_=Wkd.rearrange("(o p) n -> p o n", p=P)[:, o, :])
        for o in range(DO):
            nc.sync.dma_start(
                out=Wo[:, o, :],
                in_=Wod.rearrange("(o p) n -> p o n", p=P)[:, o, :])
        w1_r = consts.tile([1, D], BF16)
        nc.sync.dma_start(out=w1_r, in_=w1d[None, :])

        # ---------------- v projection ----------------
        # v[keys, dv] = K @ Wv + 1 (x) bv, stored per (kb, head) as
        # [v_h (64 cols) | ones (64 cols)] so the U matmul produces the
        # softmax denominator replicated over psum rows 64:128.
        vA = bigs.tile([P, nkb, H, P], BF16, tag="va")
        for kb2 in range(0, nkb, 2):
            ps = ps_a.tile([P, NQ], F32, tag="pp")
            for j, kb in enumerate(range(kb2, min(kb2 + 2, nkb))):
                half = ps[:, j * 512:(j + 1) * 512]
                for ko in range(DO):
                    nc.tensor.matmul(half,
                                     lhsT=KT[:, ko, kb * P:(kb + 1) * P],
                                     rhs=Wv[:, ko, :],
                                     start=(ko == 0), stop=False)
                nc.tensor.matmul(half, lhsT=ones_bf[:1, :P], rhs=bv_r,
                                 start=False, stop=True)
                nc.scalar.copy(
                    out=vA[:, kb, :, 0:HD],
                    in_=half.rearrange("p (h e) -> p h e", e=HD))
                nc.gpsimd.tensor_copy(
                    out=vA[:, kb, :, HD:P],
                    in_=ones_bf.rearrange("p (h e) -> p h e", e=HD))

        # -------- projections + attention (software-pipelined) --------
        qTb = bigs.tile([P, DO, NQ], BF16, tag="qproj")
        kTp = bigs.tile([P, DO, NKC], BF16, tag="kproj")
        ZT = bigs.tile([P, DO, NQ], BF16, tag="zt")

        def proj_q(g):
            ps = ps_a.tile([P, NQ], F32, tag="pp")
            for qc in range(QC):
                for ko in range(DO):
                    nc.tensor.matmul(
                        ps[:, qc * 512:(qc + 1) * 512],
                        lhsT=Wq[:, ko, g * P:(g + 1) * P],
                        rhs=QT[:, ko, qc * 512:(qc + 1) * 512],
                        start=(ko == 0), stop=(ko == DO - 1))
            nc.vector.tensor_scalar_add(
                out=qTb[:, g, :], in0=ps, scalar1=bqT[:, g:g + 1])

        def proj_k(g):
            ps = ps_a.tile([P, NQ], F32, tag="pp")
            for off, w in kchunks:
                for ko in range(DO):
                    nc.tensor.matmul(
                        ps[:, off:off + w],
                        lhsT=Wk[:, ko, g * P:(g + 1) * P],
                        rhs=KT[:, ko, off:off + w],
                        start=(ko == 0), stop=(ko == DO - 1))
            nc.vector.tensor_scalar_add(
                out=kTp[:, g, :], in0=ps[:, :NKC], scalar1=bkT[:, g:g + 1])

        ups_of = {}

        def S_emit(h, kb):
            g, hh = divmod(h, 2)
            r0 = HD * hh
            sc = ps_a.tile([P, NQ], F32, tag="pp", name=f"S{h}_{kb}")
            for qc in range(QC):
                nc.tensor.matmul(
                    sc[:, qc * 512:(qc + 1) * 512],
                    lhsT=kTp[r0:r0 + HD, g, kb * P:(kb + 1) * P],
                    rhs=qTb[r0:r0 + HD, g, qc * 512:(qc + 1) * 512],
                    start=True, stop=True)
            pt = ptp.tile([P, NQ], BF16, tag="pt")
            nc.scalar.activation(pt, sc, AF.Exp,
                                 bias=mbT[:, kb:kb + 1], scale=SCALE)
            return pt

        def U_emit(h, kb, pt):
            if kb == 0:
                ups_of[h] = ps_b.tile([P, NQ], F32, tag="u", name=f"U{h}")
            ups = ups_of[h]
            for qc in range(QC):
                nc.tensor.matmul(
                    ups[:, qc * 512:(qc + 1) * 512],
                    lhsT=vA[:, kb, h, :],
                    rhs=pt[:, qc * 512:(qc + 1) * 512],
                    start=(kb == 0), stop=(kb == nkb - 1))

        def epilogue(h):
            g, hh = divmod(h, 2)
            r0 = HD * hh
            ups = ups_of.pop(h)
            rec = recp.tile([HD, NQ], F32, tag="rec")
            nc.vector.reciprocal(out=rec, in_=ups[HD:P, :])
            dst = ZT[r0:r0 + HD, g, :]
            nc.vector.tensor_tensor(out=dst, in0=ups[0:HD, :], in1=rec,
                                    op=OP.mult)
            eng = nc.vector if h >= H - 2 else nc.gpsimd
            eng.tensor_tensor(out=dst, in0=dst,
                              in1=qTb[r0:r0 + HD, g, :], op=OP.add)

        proj_q(0)
        proj_k(0)
        pend = []

        def flush_one():
            h2, kb2, pt2 = pend.pop(0)
            U_emit(h2, kb2, pt2)
            if kb2 == nkb - 1:
                epilogue(h2)

        for h in range(H):
            for kb in range(nkb):
                if h % 2 == 0 and h < H - 2:
                    if kb == 1:
                        proj_q(h // 2 + 1)
                    elif kb == 3 or (nkb < 4 and kb == nkb - 1):
                        proj_k(h // 2 + 1)
                pend.append((h, kb, S_emit(h, kb)))
                if len(pend) > 2:
                    flush_one()
        while pend:
            flush_one()

        # ---------------- LN0 / FC / LN1 (per-qc pipelined) -----------
        def ln_stats(src):
            """src bf16 [P, DO, NQ] -> (s1, s2) psum [P, NQ] replicated."""
            sq = bigs.tile([P, DO, NQ], BF16, tag="sq")
            for qc in range(QC):
                nc.vector.tensor_tensor(
                    out=sq[:, :, qc * 512:(qc + 1) * 512],
                    in0=src[:, :, qc * 512:(qc + 1) * 512],
                    in1=src[:, :, qc * 512:(qc + 1) * 512], op=OP.mult)
            s1 = ps_b.tile([P, NQ], F32, tag="u", name="s1")
            s2 = ps_b.tile([P, NQ], F32, tag="u", name="s2")
            for qc in range(QC):
                for ko in range(DO):
                    nc.tensor.matmul(
                        s1[:, qc * 512:(qc + 1) * 512],
                        lhsT=ones_bf[:, :P],
                        rhs=src[:, ko, qc * 512:(qc + 1) * 512],
                        start=(ko == 0), stop=(ko == DO - 1))
                for ko in range(DO):
                    nc.tensor.matmul(
                        s2[:, qc * 512:(qc + 1) * 512],
                        lhsT=ones_bf[:, :P],
                        rhs=sq[:, ko, qc * 512:(qc + 1) * 512],
                        start=(ko == 0), stop=(ko == DO - 1))
            return s1, s2

        def ln_finalize(s1, s2, qc, A, C, c_is_mu_only):
            qs = slice(qc * 512, (qc + 1) * 512)
            t1 = stp.tile([P, 512], F32, tag="t1")
            nc.scalar.activation(t1, s1[:, qs], AF.Square, scale=1.0 / D)
            t2 = stp.tile([P, 512], F32, tag="t2")
            nc.vector.scalar_tensor_tensor(
                out=t2, in0=s2[:, qs], scalar=1.0 / D, in1=t1,
                op0=OP.mult, op1=OP.subtract)
            t3 = stp.tile([P, 512], F32, tag="t3")
            nc.scalar.activation(t3, t2, AF.Sqrt, bias=epsT)
            with nc.allow_low_precision("bf16 rstd is plenty for 2e-2"):
                nc.vector.reciprocal(out=A[:, qs], in_=t3)
            if c_is_mu_only:
                nc.vector.tensor_scalar_mul(out=C[:, qs], in0=s1[:, qs],
                                            scalar1=-1.0 / D)
            else:
                nc.vector.scalar_tensor_tensor(
                    out=C[:, qs], in0=s1[:, qs], scalar=-1.0 / D, in1=A[:, qs],
                    op0=OP.mult, op1=OP.mult)

        def bcast(t, qc):
            return t[:, qc * 512:(qc + 1) * 512].rearrange(
                "p (a t) -> p a t", a=1).to_broadcast((P, DO, 512))

        # ---- LN0 ----
        s1, s2 = ln_stats(ZT)
        A0 = abp.tile([P, NQ], BF16, tag="A")
        C0 = abp.tile([P, NQ], BF16, tag="C")
        XT = bigs.tile([P, DO, NQ], BF16, tag="xt")
        RT = bigs.tile([P, DO, NQ], BF16, tag="rt")
        O2 = bigs.tile([P, DO, NQ], BF16, tag="o2")
        for qc in range(QC):
            qs = slice(qc * 512, (qc + 1) * 512)
            ln_finalize(s1, s2, qc, A0, C0, c_is_mu_only=False)
            nc.vector.tensor_tensor(out=XT[:, :, qs], in0=ZT[:, :, qs],
                                    in1=bcast(A0, qc), op=OP.mult)
            if apply_g0b0:
                nc.vector.tensor_tensor(out=XT[:, :, qs], in0=XT[:, :, qs],
                                        in1=bcast(C0, qc), op=OP.add)
                for g in range(DO):
                    nc.vector.tensor_scalar(
                        out=XT[:, g, qs], in0=XT[:, g, qs],
                        scalar1=g0T[:, g:g + 1], scalar2=b0T[:, g:g + 1],
                        op0=OP.mult, op1=OP.add)
            # ---- FC on this qc chunk ----
            for g2 in range(0, DO, 2):
                fps = ps_a.tile([P, NQ], F32, tag="pp")
                for j, g in enumerate((g2, g2 + 1)):
                    half = fps[:, j * 512:(j + 1) * 512]
                    for ko in range(DO):
                        nc.tensor.matmul(
                            half, lhsT=Wo[:, ko, g * P:(g + 1) * P],
                            rhs=XT[:, ko, qs],
                            start=(ko == 0),
                            stop=apply_g0b0 and ko == DO - 1)
                    if not apply_g0b0:
                        # rank-1: + w1 (x) C0   (the -mu*rstd shift of LN0)
                        nc.tensor.matmul(
                            half, lhsT=w1_r[:1, g * P:(g + 1) * P],
                            rhs=C0[0:1, qs], start=False, stop=True)
                    nc.vector.tensor_scalar(
                        out=RT[:, g, qs], in0=half,
                        scalar1=boT[:, g:g + 1], scalar2=0.0,
                        op0=OP.add, op1=OP.max)
            nc.vector.tensor_tensor(out=O2[:, :, qs], in0=XT[:, :, qs],
                                    in1=RT[:, :, qs], op=OP.add)

        # ---- LN1 ----
        # LN1 absorbs the uniform per-token shift C0 left out of the
        # residual stream (layernorm is shift-invariant per token).
        s1b, s2b = ln_stats(O2)
        A1 = abp.tile([P, NQ], BF16, tag="A")
        C1 = abp.tile([P, NQ], BF16, tag="C")
        TA = bigs.tile([P, DO, NQ], BF16, tag="ta")
        OTb = bigs.tile([P, DO, NQ], BF16, tag="otb")
        OTv = OTd.rearrange("(g p) t -> p g t", p=P)
        for qc in range(QC):
            qs = slice(qc * 512, (qc + 1) * 512)
            ln_finalize(s1b, s2b, qc, A1, C1, c_is_mu_only=True)
            nc.gpsimd.tensor_tensor(out=TA[:, :, qs], in0=O2[:, :, qs],
                                    in1=bcast(C1, qc), op=OP.add)
            nc.vector.tensor_tensor(out=OTb[:, :, qs], in0=TA[:, :, qs],
                                    in1=bcast(A1, qc), op=OP.mult)
            if apply_g1b1:
                for g in range(DO):
                    nc.vector.tensor_scalar(
                        out=OTb[:, g, qs], in0=OTb[:, g, qs],
                        scalar1=g1T[:, g:g + 1], scalar2=b1T[:, g:g + 1],
                        op0=OP.mult, op1=OP.add)
            for g in range(DO):
                nc.sync.dma_start(out=OTv[:, g, qs], in_=OTb[:, g, qs])


# ------------------------------------------------------------------
# host-side entry point
# ------------------------------------------------------------------
_CACHE = {}


def _get_program(repeat, apply_g0b0, apply_g1b1, nkb=NO):
    key = (repeat, apply_g0b0, apply_g1b1, nkb)
    if key not in _CACHE:
        _CACHE[key] = build_program(repeat, apply_g0b0, apply_g1b1, nkb)
    return _CACHE[key]


def compact_keys(K_b, mask_b, nkb):
    """Move unmasked keys to the front (order-preserving) and truncate to
    nkb*128 rows.  Softmax over keys is permutation-invariant and fully
    masked keys contribute exactly zero, so this is output-preserving as
    long as all unmasked keys survive the truncation."""
    nkc = nkb * P
    order = np.argsort(mask_b == 0, kind="stable")[:nkc]
    return (np.ascontiguousarray(K_b[order]),
            np.ascontiguousarray(mask_b[order]))


def pick_nkb(attention_mask):
    counts = (np.asarray(attention_mask) != 0).sum(axis=-1)
    return max(1, min(NO, int(-(-int(counts.max()) // P))))


def make_in_maps(inputs, nkb):
    f32 = np.float32
    Wq = np.asarray(inputs["Wq"], f32)
    Wk = np.asarray(inputs["Wk"], f32)
    Wv = np.asarray(inputs["Wv"], f32)
    Wo = np.asarray(inputs["Wo"], f32)
    g0 = np.asarray(inputs["g0"], f32)
    b0 = np.asarray(inputs["b0"], f32)
    g1 = np.asarray(inputs["g1"], f32)
    b1 = np.asarray(inputs["b1"], f32)
    apply_g0b0 = not (np.all(g0 == 1.0) and np.all(b0 == 0.0))
    apply_g1b1 = not (np.all(g1 == 1.0) and np.all(b1 == 0.0))

    # w1 is only consumed by the rank-1 C0-fold, which is disabled when
    # apply_g0b0 (the full LN0 output is materialized instead).
    w1 = Wo.sum(axis=0)
    bo = np.asarray(inputs["bo"], f32)

    shared = {
        "Wq": Wq.astype(NP_BF16), "Wk": Wk.astype(NP_BF16),
        "Wv": Wv.astype(NP_BF16), "Wo": Wo.astype(NP_BF16),
        "bvb": np.asarray(inputs["bv"], f32).astype(NP_BF16),
        "w1": w1.astype(NP_BF16),
    }

    def cols_for(mask_b):
        mb = np.where(mask_b != 0, 0.0, -100.0).astype(f32)
        parts = [np.asarray(inputs["bq"], f32).reshape(4, P).T,
                 np.asarray(inputs["bk"], f32).reshape(4, P).T,
                 bo.reshape(4, P).T,
                 mb.reshape(nkb, P).T]
        if apply_g0b0:
            parts += [g0.reshape(4, P).T, b0.reshape(4, P).T]
        if apply_g1b1:
            parts += [g1.reshape(4, P).T, b1.reshape(4, P).T]
        return np.ascontiguousarray(np.concatenate(parts, axis=1))

    Q = np.asarray(inputs["Q"], f32)
    K = np.asarray(inputs["K"], f32)
    mask = np.asarray(inputs["attention_mask"], np.int32)
    in_maps = []
    for b in range(B):
        m = dict(shared)
        Kc, mc = compact_keys(K[b], mask[b], nkb)
        m["Q"] = np.ascontiguousarray(Q[b]).astype(NP_BF16)
        m["K"] = Kc.astype(NP_BF16)
        m["COLS"] = cols_for(mc)
        in_maps.append(m)
    return in_maps


def kernel(Q, K, attention_mask, Wq, bq, Wk, bk, Wv, bv, Wo, bo,
           g0, b0, g1, b1, _repeat=1):
    inputs = {
        "Q": Q, "K": K, "attention_mask": attention_mask,
        "Wq": Wq, "bq": bq, "Wk": Wk, "bk": bk, "Wv": Wv, "bv": bv,
        "Wo": Wo, "bo": bo, "g0": g0, "b0": b0, "g1": g1, "b1": b1,
    }
    apply_g0b0 = not (np.all(np.asarray(g0) == 1.0)
                      and np.all(np.asarray(b0) == 0.0))
    apply_g1b1 = not (np.all(np.asarray(g1) == 1.0)
                      and np.all(np.asarray(b1) == 0.0))
    nkb = pick_nkb(attention_mask)
    nc = _get_program(_repeat, apply_g0b0, apply_g1b1, nkb)
    in_maps = make_in_maps(inputs, nkb)

    res = bass_utils.run_bass_kernel_spmd(
        nc, in_maps, core_ids=list(range(N_CORES)), trace=False)

    out = np.empty((B, NQ, D), np.float32)
    for b in range(B):
        out[b] = res.results[b]["OT"].astype(np.float32).T
    return out
